# revision 47
# baseline (speedup 1.0000x reference)
"""Ising-model energy kernel for 8 Trainium2 NeuronCores (fp8 streaming).

result = 0.25*S0 - 0.5*(Qup + Qdiag + S2)
  S0    = sum(A)                      (A = info_mtx, 8192x8192 fp32)
  Qup   = sum_{i<j, tile(j)>tile(i)} A[i,j] s_i s_j   (device)
  Qdiag = intra-diagonal-tile strict-upper terms       (host, f64)
  S2    = sum_i A[i,i] s_i                             (host, f64)

Device strategy (per core, row-shard 1024x8192):
  A is quantized to fp8e4m3 on host (answer tolerance 2e-2; fp8 end-to-end
  error ~1e-5) which cuts HBM traffic 4x vs fp32; the stream then runs at
  the ~358 GB/s HBM-per-core line rate, which is the roofline.  The state
  vector is the STATIONARY operand: each matmul is stationary [128,2] =
  [s_block*mask, 1] x moving A-chunk [128,512], accumulating [2,512] in
  PSUM: row 0 = tile-masked partial matvec, row 1 = column sums.  8192
  columns = 16 chunks, processed as 4 quartets with 4-way PE column
  tiling: chunk 4q+c lands on PSUM bank q partitions [32c, 32c+2), so the
  four moving streams run concurrently on separate PE column stripes and
  no PSUM bank is ever reused (no WAR waits - walrus allows one sync wait
  per instruction).  The strict upper-tile mask is all-or-none per
  (block, chunk) except one partial chunk per block (extra sub-range
  matmul); diagonal 128x128 tiles go to the host.  Mask values live in
  the stationary data (w tensor), and a per-core chunk->slot permutation
  puts each core's partial chunks at slots 0-1 so the instruction stream
  is identical on all 8 cores (SPMD) while the mask geometry differs.
  Each 2MB group is DMAed as SUB sub-slabs so the PE starts early and
  trails the last byte by only one round.  A fused DVE multiply-reduce
  against a staged [s, 1] table collapses each quartet's PSUM bank to
  [98,1] scalars on-device, so the kernel tail is one tiny SWDGE DMA.
"""

import numpy as np
import ml_dtypes

N = 8192
NCORES = 8
ROWS = N // NCORES       # 1024 rows per core
BLK = 128                # partition block
NB = ROWS // BLK         # 8 row blocks per core
CW = 512                 # chunk width = one PSUM bank of fp32
NCH = N // CW            # 16 column chunks (= device slots)
CPG = 4                  # chunks per group = col-tiling width
G = NCH // CPG           # 4 groups (quartets)
SUB = 4                  # sub-DMAs per group
NBS = NB // SUB          # blocks per sub-slab
SFREE = CPG * NBS * CW   # values per partition per sub-slab
NW = 2 * NCH * NB + 2 * 6  # 268 stationary columns (main pairs + extras)

F8 = ml_dtypes.float8_e4m3

_NC_CACHE = None
LAST_EXEC_NS = None
LAST_RESULTS = None


def _perm(d):
    """Chunk processed at slot i is perm[i].  Partial chunks (2d, 2d+1)
    always sit at slots 0-1 so the device program is core-independent."""
    head = [2 * d, 2 * d + 1]
    return head + [c for c in range(NCH) if c not in head]


def _stripe_ops(q, c):
    """Op list for PSUM stripe c of quartet q: (w_pair_col, block, off, len).
    Core-independent by construction."""
    slot = CPG * q + c
    ops = [(2 * (slot * NB + b), b, 0, CW) for b in range(NB)]
    if slot < 2:
        for bb in range(3):
            b = slot * 4 + bb          # slot0: b=0,1,2   slot1: b=4,5,6
            off = (b % 4 + 1) * BLK
            ops.append((2 * NCH * NB + 2 * (slot * 3 + bb), b, off, CW - off))
    return ops


def _build_nc():
    import concourse.bass as bass
    import concourse.tile as tile
    from concourse.tile_rust import add_dep_helper
    from concourse import mybir

    f8 = mybir.dt.float8e4
    f32 = mybir.dt.float32
    nc = bass.Bass()
    a = nc.dram_tensor("a", [G * SUB, BLK, SFREE], f8, kind="ExternalInput")
    w = nc.dram_tensor("w", [BLK, NW], f8, kind="ExternalInput")
    sc = nc.dram_tensor("sc", [2, CPG, G * CW], f32, kind="ExternalInput")
    o = nc.dram_tensor("o", [98, G], f32, kind="ExternalOutput")

    with tile.TileContext(nc) as tc:
        with (
            tc.tile_pool(name="slab", bufs=G * SUB) as slab_pool,
            tc.tile_pool(name="small", bufs=1) as small,
            tc.tile_pool(name="scr", bufs=G) as scr_pool,
            tc.tile_pool(name="psum", bufs=G, space="PSUM") as psum_pool,
            tc.tile_pool(name="warmp", bufs=1, space="PSUM") as warm_pool,
        ):
            wt = small.tile([BLK, NW], f8)
            sct = small.tile([98, G * CW], f32)
            # HAM warm-up data: zero before any ACT DMA dispatches so the
            # dummy matmuls can start as soon as the engines come up.
            bf16 = mybir.dt.bfloat16
            warm_w = small.tile([BLK, 2], bf16)
            warm_mv = small.tile([BLK, CW], bf16)
            nc.scalar.memzero(warm_w[:, :])
            nc.scalar.memzero(warm_mv[:, :])
            sc_dmas = [
                nc.scalar.dma_start(out=sct[0:98:32, :], in_=sc[0, :, :]),
                nc.scalar.dma_start(out=sct[1:98:32, :], in_=sc[1, :, :]),
            ]
            loads = [nc.scalar.dma_start(out=wt, in_=w[:, :])] + sc_dmas
            # DVE fences absorb the sct load waits so the first quartet's
            # TensorTensor keeps only its PE wait (walrus allows one sync
            # wait per instruction).
            for dma in sc_dmas:
                fence = nc.vector.engine_nop()
                add_dep_helper(fence.ins, dma.ins, sync=True, reason="sct fence")
            outsc = small.tile([98, G], f32)
            # HAM warm-up: the PE clock-gate defaults to 4/8 (1.2 GHz) and
            # only reaches 2.4 GHz after ~3.4us of sustained activity.  Fill
            # the dead window while the first slab DMA is in flight with
            # dummy matmuls so the real matmuls run warm.
            warm_p = warm_pool.tile([BLK, CW], f32)
            for _ in range(6):
                nc.tensor.matmul(
                    warm_p[0:2, :], warm_w[:, :], warm_mv[:, :],
                    start=True, stop=True,
                )
            last_mm = last_red = None
            for q in range(G):
                subs = []
                for k in range(SUB):
                    sl = slab_pool.tile([BLK, SFREE], f8)
                    loads.append(nc.sync.dma_start(out=sl, in_=a[q * SUB + k, :, :]))
                    subs.append(sl)
                P = psum_pool.tile([BLK, CW], f32)
                # round-major interleave of the 4 stripes' op lists
                stripes = [_stripe_ops(q, c) for c in range(CPG)]
                nops = max(len(x) for x in stripes)
                for i in range(nops):
                    # Keep the HAM clock-gate warm: dummy matmuls fill the
                    # PE's DMA-wait gaps with activity (so rounds run at
                    # 2.4 GHz, not 1.2).  Every round in early quartets;
                    # only sub-boundary rounds in the last quartet so the
                    # dummies sit inside the receipt-wait windows and stay
                    # off the tail critical path.
                    if i < NB and (q < G - 1 or i % NBS == 0):
                        nc.tensor.matmul(
                            warm_p[0:2, :], warm_w[:, :], warm_mv[:, :],
                            start=True, stop=True,
                        )
                    for c in range(CPG):
                        if i >= len(stripes[c]):
                            continue
                        wc, b, off, ln = stripes[c][i]
                        sl = subs[b // NBS]
                        base = (c * NBS + b % NBS) * CW + off
                        last_mm = nc.tensor.matmul(
                            P[32 * c : 32 * c + 2, off : off + ln],
                            wt[:, wc : wc + 2],
                            sl[:, base : base + ln],
                            start=(i == 0),
                            stop=(i == len(stripes[c]) - 1),
                            tile_position=(0, 32 * c),
                        )
                # DVE reduce: outsc[:, q] = sum_j P[:, j] * sct[:, q*CW+j]
                # (row 32c = matvec . s_chunk, row 32c+1 = colsum total;
                # other partitions are garbage and ignored by the host).
                # Per-quartet scratch: tile reuse would add a second sync
                # wait to TensorTensor (walrus allows one).
                scratch = scr_pool.tile([98, CW], f32)
                last_red = nc.vector.tensor_tensor_reduce(
                    out=scratch[:, :],
                    in0=P[0:98, :],
                    in1=sct[:, q * CW : (q + 1) * CW],
                    scale=1.0,
                    scalar=0.0,
                    op0=mybir.AluOpType.mult,
                    op1=mybir.AluOpType.add,
                    accum_out=outsc[:, q : q + 1],
                    opt_aps=False,
                )
            # SWDGE path: DMASW lanes are otherwise unused, so this gets a
            # fresh completion lane and carries only its single data wait
            # (walrus allows one sync wait per DMA instruction).
            out_dmas = [nc.gpsimd.dma_start(out=o[:, :], in_=outsc[:, :])]
            # The kernel-tail flush drain lands on SP and would aggregate
            # every outstanding sem into one multi-wait instruction (walrus
            # allows one wait clause).  These 1-wait SP nops make SP observe
            # each sem individually so the drain has nothing left to wait on.
            for dep in loads + [last_mm, last_red] + out_dmas:
                nop = nc.sync.nop()
                add_dep_helper(nop.ins, dep.ins, sync=True, reason="tail sem absorb")
    return nc


def _prep_inputs(A, s):
    """Per-core in_maps (fp8 slab groups + stationary data + s table)."""
    s_blocks = s.reshape(N // BLK, BLK)
    in_maps = []
    for d in range(NCORES):
        perm = _perm(d)
        A8 = (
            A[d * ROWS : (d + 1) * ROWS]
            .astype(F8)
            .reshape(NB, BLK, NCH, CW)
        )
        T = A8.transpose(2, 1, 0, 3)[perm]  # [slot, p, b, j]
        ag = np.ascontiguousarray(
            T.reshape(G, CPG, BLK, SUB, NBS, CW)
            .transpose(0, 3, 1, 4, 2, 5)    # [G, SUB, CPG, NBS, BLK, CW]
            .transpose(0, 1, 4, 2, 3, 5)    # [G, SUB, BLK, CPG, NBS, CW]
            .reshape(G * SUB, BLK, SFREE)
        )

        wmat = np.zeros((BLK, NW), np.float32)
        for slot in range(NCH):
            cc = perm[slot]
            for b in range(NB):
                Bg = NB * d + b
                if cc > Bg // 4:  # chunk fully above this block's diagonal
                    wmat[:, 2 * (slot * NB + b)] = s_blocks[Bg]
                wmat[:, 2 * (slot * NB + b) + 1] = 1.0
        for slot in range(2):
            for bb in range(3):
                b = slot * 4 + bb
                wmat[:, 2 * NCH * NB + 2 * (slot * 3 + bb)] = s_blocks[NB * d + b]

        # DVE reduce table: row r=0 carries s for the chunk at each slot,
        # row r=1 carries ones (colsum pass-through)
        scm = np.zeros((2, CPG, G * CW), np.float32)
        for q in range(G):
            for c in range(CPG):
                cc = perm[CPG * q + c]
                scm[0, c, q * CW : (q + 1) * CW] = s[cc * CW : (cc + 1) * CW]
                scm[1, c, q * CW : (q + 1) * CW] = 1.0
        in_maps.append({"a": ag, "w": wmat.astype(F8), "sc": scm})
    return in_maps


def _sim_core(in_map):
    """Numpy replica of the device program (for layout validation)."""
    out = np.zeros((98, G), np.float32)
    agf = in_map["a"].astype(np.float32)
    wf = in_map["w"].astype(np.float32)
    scm = in_map["sc"]
    for q in range(G):
        for c in range(CPG):
            acc = np.zeros((2, CW), np.float32)
            for wc, b, off, ln in _stripe_ops(q, c):
                base = (c * NBS + b % NBS) * CW + off
                mov = agf[q * SUB + b // NBS][:, base : base + ln]
                acc[:, off : off + ln] += wf[:, wc : wc + 2].T @ mov
            for r in range(2):
                out[32 * c + r, q] = acc[r] @ scm[r, c, q * CW : (q + 1) * CW]
    return out


def _postprocess(A, s, outs):
    s64 = s.astype(np.float64)
    s_blocks = s.reshape(N // BLK, BLK)
    S0 = 0.0
    Qup = 0.0
    for d in range(NCORES):
        out = outs[d].astype(np.float64)  # [98, G]
        for c in range(CPG):
            Qup += out[32 * c, :].sum()
            S0 += out[32 * c + 1, :].sum()
    Qdiag = 0.0
    for Bg in range(N // BLK):
        blk = A[Bg * BLK : (Bg + 1) * BLK, Bg * BLK : (Bg + 1) * BLK].astype(np.float64)
        sb = s_blocks[Bg].astype(np.float64)
        Qdiag += sb @ (np.triu(blk, 1) @ sb)
    S2 = float(np.diagonal(A).astype(np.float64) @ s64)
    return 0.25 * S0 - 0.5 * (Qup + Qdiag + S2)


def kernel(info_mtx: np.ndarray, state: np.ndarray, _trace: bool = False, _sim: bool = False) -> np.ndarray:
    global _NC_CACHE, LAST_EXEC_NS, LAST_RESULTS

    A = np.ascontiguousarray(np.asarray(info_mtx, dtype=np.float32))
    s = np.ascontiguousarray(np.asarray(state, dtype=np.float32))
    in_maps = _prep_inputs(A, s)

    if _sim:
        outs = [_sim_core(m) for m in in_maps]
        return np.asarray(_postprocess(A, s, outs), dtype=np.float32)

    if _NC_CACHE is None:
        _NC_CACHE = _build_nc()
    from concourse.bass_utils import run_bass_kernel_spmd

    res = run_bass_kernel_spmd(_NC_CACHE, in_maps, list(range(NCORES)), trace=_trace)
    LAST_EXEC_NS = res.exec_time_ns
    LAST_RESULTS = res

    outs = [res.results[d]["o"] for d in range(NCORES)]
    return np.asarray(_postprocess(A, s, outs), dtype=np.float32)


# revision 54
# speedup vs baseline: 1.0582x; 1.0582x over previous
"""Ising-model energy kernel for 8 Trainium2 NeuronCores (fp8 streaming).

result = 0.25*S0 - 0.5*(Qup + Qdiag + S2)
  S0    = sum(A)                      (A = info_mtx, 8192x8192 fp32)
  Qup   = sum_{i<j, tile(j)>tile(i)} A[i,j] s_i s_j   (device)
  Qdiag = intra-diagonal-tile strict-upper terms       (host, f64)
  S2    = sum_i A[i,i] s_i                             (host, f64)

Device strategy (per core, row-shard 1024x8192):
  A is quantized to fp8e4m3 on host (answer tolerance 2e-2; fp8 end-to-end
  error ~1e-5) which cuts HBM traffic 4x vs fp32; the stream then runs at
  the ~358 GB/s HBM-per-core line rate, which is the roofline.  The state
  vector is the STATIONARY operand: each matmul is stationary [128,2] =
  [s_block*mask, 1] x moving A-chunk [128,512], accumulating [2,512] in
  PSUM: row 0 = tile-masked partial matvec, row 1 = column sums.  8192
  columns = 16 chunks, processed as 4 quartets with 4-way PE column
  tiling: chunk 4q+c lands on PSUM bank q partitions [32c, 32c+2), so the
  four moving streams run concurrently on separate PE column stripes and
  no PSUM bank is ever reused (no WAR waits - walrus allows one sync wait
  per instruction).  The strict upper-tile mask is all-or-none per
  (block, chunk) except one partial chunk per block (extra sub-range
  matmul); diagonal 128x128 tiles go to the host.  Mask values live in
  the stationary data (w tensor), and a per-core chunk->slot permutation
  puts each core's partial chunks at slots 0-1 so the instruction stream
  is identical on all 8 cores (SPMD) while the mask geometry differs.
  Each 2MB group is DMAed as SUB sub-slabs so the PE starts early and
  trails the last byte by only one round.  A fused DVE multiply-reduce
  against a staged [s, 1] table collapses each quartet's PSUM bank to
  [98,1] scalars on-device, so the kernel tail is one tiny SWDGE DMA.
"""

import numpy as np
import ml_dtypes

N = 8192
NCORES = 8
ROWS = N // NCORES       # 1024 rows per core
BLK = 128                # partition block
NB = ROWS // BLK         # 8 row blocks per core
CW = 512                 # chunk width = one PSUM bank of fp32
NCH = N // CW            # 16 column chunks (= device slots)
CPG = 4                  # chunks per group = col-tiling width
G = NCH // CPG           # 4 groups (quartets)
SUB = 4                  # sub-DMAs per group
NBS = NB // SUB          # blocks per sub-slab
SFREE = CPG * NBS * CW   # values per partition per sub-slab
NW = 2 * NCH * NB + 2 * 6  # 268 stationary columns (main pairs + extras)

F8 = ml_dtypes.float8_e4m3

_NC_CACHE = None
LAST_EXEC_NS = None
LAST_RESULTS = None


def _perm(d):
    """Chunk processed at slot i is perm[i].  Partial chunks (2d, 2d+1)
    always sit at slots 0-1 so the device program is core-independent."""
    head = [2 * d, 2 * d + 1]
    return head + [c for c in range(NCH) if c not in head]


def _stripe_ops(q, c):
    """Op list for PSUM stripe c of quartet q: (w_pair_col, block, off, len).
    Core-independent by construction."""
    slot = CPG * q + c
    ops = [(2 * (slot * NB + b), b, 0, CW) for b in range(NB)]
    if slot < 2:
        for bb in range(3):
            b = slot * 4 + bb          # slot0: b=0,1,2   slot1: b=4,5,6
            off = (b % 4 + 1) * BLK
            ops.append((2 * NCH * NB + 2 * (slot * 3 + bb), b, off, CW - off))
    return ops


def _build_nc():
    import concourse.bass as bass
    import concourse.tile as tile
    from concourse.tile_rust import add_dep_helper
    from concourse import mybir

    f8 = mybir.dt.float8e4
    f32 = mybir.dt.float32
    nc = bass.Bass()
    a = nc.dram_tensor("a", [G * SUB, BLK, SFREE], f8, kind="ExternalInput")
    w = nc.dram_tensor("w", [BLK, NW], f8, kind="ExternalInput")
    sc = nc.dram_tensor("sc", [2, CPG, G * CW], f32, kind="ExternalInput")
    o = nc.dram_tensor("o", [98, G], f32, kind="ExternalOutput")

    with tile.TileContext(nc) as tc:
        with (
            tc.tile_pool(name="slab", bufs=G * SUB) as slab_pool,
            tc.tile_pool(name="small", bufs=1) as small,
            tc.tile_pool(name="scr", bufs=G) as scr_pool,
            tc.tile_pool(name="psum", bufs=G, space="PSUM") as psum_pool,
            tc.tile_pool(name="warmp", bufs=1, space="PSUM") as warm_pool,
        ):
            wt = small.tile([BLK, NW], f8)
            sct = small.tile([98, G * CW], f32)
            # HAM warm-up data: zero before any ACT DMA dispatches so the
            # dummy matmuls can start as soon as the engines come up.
            bf16 = mybir.dt.bfloat16
            warm_w = small.tile([BLK, 2], bf16)
            warm_mv = small.tile([BLK, CW], bf16)
            nc.scalar.memzero(warm_w[:, :])
            nc.scalar.memzero(warm_mv[:, :])
            sc_dmas = [
                nc.scalar.dma_start(out=sct[0:98:32, :], in_=sc[0, :, :]),
                nc.scalar.dma_start(out=sct[1:98:32, :], in_=sc[1, :, :]),
            ]
            loads = [nc.scalar.dma_start(out=wt, in_=w[:, :])] + sc_dmas
            # DVE fences absorb the sct load waits so the first quartet's
            # TensorTensor keeps only its PE wait (walrus allows one sync
            # wait per instruction).
            for dma in sc_dmas:
                fence = nc.vector.engine_nop()
                add_dep_helper(fence.ins, dma.ins, sync=True, reason="sct fence")
            outsc = small.tile([98, G], f32)
            # HAM warm-up: the PE clock-gate defaults to 4/8 (1.2 GHz) and
            # only reaches 2.4 GHz after ~3.4us of sustained activity.  Fill
            # the dead window while the first slab DMA is in flight with
            # dummy matmuls so the real matmuls run warm.
            warm_p = warm_pool.tile([BLK, CW], f32)
            for _ in range(6):
                nc.tensor.matmul(
                    warm_p[0:2, :], warm_w[:, :], warm_mv[:, :],
                    start=True, stop=True,
                )
            last_mm = last_red = None
            for q in range(G):
                subs = []
                for k in range(SUB):
                    sl = slab_pool.tile([BLK, SFREE], f8)
                    loads.append(nc.sync.dma_start(out=sl, in_=a[q * SUB + k, :, :]))
                    subs.append(sl)
                P = psum_pool.tile([BLK, CW], f32)
                # round-major interleave of the 4 stripes' op lists
                stripes = [_stripe_ops(q, c) for c in range(CPG)]
                nops = max(len(x) for x in stripes)
                for i in range(nops):
                    # Keep the HAM clock-gate warm: dummy matmuls fill the
                    # PE's DMA-wait gaps with activity (so rounds run at
                    # 2.4 GHz, not 1.2).  Every round in early quartets;
                    # only sub-boundary rounds in the last quartet so the
                    # dummies sit inside the receipt-wait windows and stay
                    # off the tail critical path.
                    if i < NB and (q < G - 1 or i % NBS == 0):
                        nc.tensor.matmul(
                            warm_p[0:2, :], warm_w[:, :], warm_mv[:, :],
                            start=True, stop=True,
                        )
                    for c in range(CPG):
                        if i >= len(stripes[c]):
                            continue
                        wc, b, off, ln = stripes[c][i]
                        sl = subs[b // NBS]
                        base = (c * NBS + b % NBS) * CW + off
                        last_mm = nc.tensor.matmul(
                            P[32 * c : 32 * c + 2, off : off + ln],
                            wt[:, wc : wc + 2],
                            sl[:, base : base + ln],
                            start=(i == 0),
                            stop=(i == len(stripes[c]) - 1),
                            tile_position=(0, 32 * c),
                        )
                # DVE reduce: outsc[:, q] = sum_j P[:, j] * sct[:, q*CW+j]
                # (row 32c = matvec . s_chunk, row 32c+1 = colsum total;
                # other partitions are garbage and ignored by the host).
                # Per-quartet scratch: tile reuse would add a second sync
                # wait to TensorTensor (walrus allows one).
                scratch = scr_pool.tile([98, CW], f32)
                last_red = nc.vector.tensor_tensor_reduce(
                    out=scratch[:, :],
                    in0=P[0:98, :],
                    in1=sct[:, q * CW : (q + 1) * CW],
                    scale=1.0,
                    scalar=0.0,
                    op0=mybir.AluOpType.mult,
                    op1=mybir.AluOpType.add,
                    accum_out=outsc[:, q : q + 1],
                    opt_aps=False,
                )
            # SWDGE path: DMASW lanes are otherwise unused, so this gets a
            # fresh completion lane and carries only its single data wait
            # (walrus allows one sync wait per DMA instruction).
            out_dmas = [nc.gpsimd.dma_start(out=o[:, :], in_=outsc[:, :])]
            # The kernel-tail flush drain lands on SP and would aggregate
            # every outstanding sem into one multi-wait instruction (walrus
            # allows one wait clause).  These 1-wait SP nops make SP observe
            # each sem individually so the drain has nothing left to wait on.
            for dep in loads + [last_mm, last_red] + out_dmas:
                nop = nc.sync.nop()
                add_dep_helper(nop.ins, dep.ins, sync=True, reason="tail sem absorb")
    return nc


def _prep_inputs(A, s):
    """Per-core in_maps (fp8 slab groups + stationary data + s table)."""
    s_blocks = s.reshape(N // BLK, BLK)
    in_maps = []
    for d in range(NCORES):
        perm = _perm(d)
        A8 = (
            A[d * ROWS : (d + 1) * ROWS]
            .astype(F8)
            .reshape(NB, BLK, NCH, CW)
        )
        T = A8.transpose(2, 1, 0, 3)[perm]  # [slot, p, b, j]
        ag = np.ascontiguousarray(
            T.reshape(G, CPG, BLK, SUB, NBS, CW)
            .transpose(0, 3, 1, 4, 2, 5)    # [G, SUB, CPG, NBS, BLK, CW]
            .transpose(0, 1, 4, 2, 3, 5)    # [G, SUB, BLK, CPG, NBS, CW]
            .reshape(G * SUB, BLK, SFREE)
        )

        wmat = np.zeros((BLK, NW), np.float32)
        for slot in range(NCH):
            cc = perm[slot]
            for b in range(NB):
                Bg = NB * d + b
                if cc > Bg // 4:  # chunk fully above this block's diagonal
                    wmat[:, 2 * (slot * NB + b)] = s_blocks[Bg]
                wmat[:, 2 * (slot * NB + b) + 1] = 1.0
        for slot in range(2):
            for bb in range(3):
                b = slot * 4 + bb
                wmat[:, 2 * NCH * NB + 2 * (slot * 3 + bb)] = s_blocks[NB * d + b]

        # DVE reduce table: row r=0 carries s for the chunk at each slot,
        # row r=1 carries ones (colsum pass-through)
        scm = np.zeros((2, CPG, G * CW), np.float32)
        for q in range(G):
            for c in range(CPG):
                cc = perm[CPG * q + c]
                scm[0, c, q * CW : (q + 1) * CW] = s[cc * CW : (cc + 1) * CW]
                scm[1, c, q * CW : (q + 1) * CW] = 1.0
        in_maps.append({"a": ag, "w": wmat.astype(F8), "sc": scm})
    return in_maps


def _sim_core(in_map):
    """Numpy replica of the device program (for layout validation)."""
    out = np.zeros((98, G), np.float32)
    agf = in_map["a"].astype(np.float32)
    wf = in_map["w"].astype(np.float32)
    scm = in_map["sc"]
    for q in range(G):
        for c in range(CPG):
            acc = np.zeros((2, CW), np.float32)
            for wc, b, off, ln in _stripe_ops(q, c):
                base = (c * NBS + b % NBS) * CW + off
                mov = agf[q * SUB + b // NBS][:, base : base + ln]
                acc[:, off : off + ln] += wf[:, wc : wc + 2].T @ mov
            for r in range(2):
                out[32 * c + r, q] = acc[r] @ scm[r, c, q * CW : (q + 1) * CW]
    return out


def _postprocess(A, s, outs):
    s64 = s.astype(np.float64)
    s_blocks = s.reshape(N // BLK, BLK)
    S0 = 0.0
    Qup = 0.0
    for d in range(NCORES):
        out = outs[d].astype(np.float64)  # [98, G]
        for c in range(CPG):
            Qup += out[32 * c, :].sum()
            S0 += out[32 * c + 1, :].sum()
    Qdiag = 0.0
    for Bg in range(N // BLK):
        blk = A[Bg * BLK : (Bg + 1) * BLK, Bg * BLK : (Bg + 1) * BLK].astype(np.float64)
        sb = s_blocks[Bg].astype(np.float64)
        Qdiag += sb @ (np.triu(blk, 1) @ sb)
    S2 = float(np.diagonal(A).astype(np.float64) @ s64)
    return 0.25 * S0 - 0.5 * (Qup + Qdiag + S2)


def kernel(info_mtx: np.ndarray, state: np.ndarray, _trace: bool = False, _sim: bool = False) -> np.ndarray:
    global _NC_CACHE, LAST_EXEC_NS, LAST_RESULTS

    A = np.ascontiguousarray(np.asarray(info_mtx, dtype=np.float32))
    s = np.ascontiguousarray(np.asarray(state, dtype=np.float32))
    in_maps = _prep_inputs(A, s)

    if _sim:
        outs = [_sim_core(m) for m in in_maps]
        return np.asarray(_postprocess(A, s, outs), dtype=np.float32)

    if _NC_CACHE is None:
        _NC_CACHE = _build_nc()
    from concourse.bass_utils import run_bass_kernel_spmd

    res = run_bass_kernel_spmd(_NC_CACHE, in_maps, list(range(NCORES)), trace=_trace)
    LAST_EXEC_NS = res.exec_time_ns
    LAST_RESULTS = res

    outs = [res.results[d]["o"] for d in range(NCORES)]
    return np.asarray(_postprocess(A, s, outs), dtype=np.float32)
